# revision 1
# baseline (speedup 1.0000x reference)
"""Discounted cumsum (B,H,S,D)=(8,16,4096,128), gamma per head, scan along S.

Strategy: batch-parallel across 8 NeuronCores (1 batch each, all 16 heads).
Per head, a two-level chunked scan implemented with PE matmuls (f32r):
  - block size T=128 along S -> 32 blocks per head, processed 4-at-a-time
    (tiles of [128 part = row-in-block, 512 free = 4 blocks x 128 d]).
  - s_k = w^T X_k   (block discounted sums)       [8 matmuls, N=512]
  - c   = AB @ s    (block-level scan, 32x32)     [1 matmul]
  - Y_k = A @ X_k + gvec (x) c_k                  [8+8 matmuls, N=512]
All matmul operands are float32r (1 cyc/row at N>=512); accumulation fp32.
"""
import sys

sys.path.insert(0, "/opt/trn_rl_repo")
import numpy as np

B, H, S, D = 8, 16, 4096, 128
T = 128          # block length along S
KB = S // T      # 32 blocks per head
TILE = 4 * T     # 512 free columns = 4 blocks per matmul
NT = S // TILE   # 8 tiles per head
SKEW_C = 1       # head-pipeline skew for the carry stage
SKEW_B = 3       # head-pipeline skew for the output stage

_CACHE = {}


def _build(repeat=1, mode="full"):
    import contextlib

    import concourse.bacc as bacc
    import concourse.tile as tile
    from concourse import mybir

    f32 = mybir.dt.float32
    f32r = mybir.dt.float32r

    nc = bacc.Bacc("TRN2", target_bir_lowering=False, debug=False)

    x_in = nc.declare_dram_parameter("x", [H, S, D], f32r, isOutput=False)
    at_in = nc.declare_dram_parameter("at", [T, H * T], f32r, isOutput=False)
    w_in = nc.declare_dram_parameter("w", [T, H], f32r, isOutput=False)
    gv_in = nc.declare_dram_parameter("gv", [1, H * T], f32r, isOutput=False)
    abt_in = nc.declare_dram_parameter("abt", [KB, H * KB], f32r, isOutput=False)
    y_out = nc.declare_dram_parameter("y", [H, S, D], f32, isOutput=True)

    with tile.TileContext(nc) as tc:
        with (
            tc.tile_pool(name="const", bufs=1) as const_pool,
            tc.tile_pool(name="xp", bufs=5) as x_pool,
            tc.tile_pool(name="op", bufs=2) as out_pool,
            tc.tile_pool(name="small", bufs=3) as small_pool,
            tc.tile_pool(name="sstage", bufs=2) as sstage_pool,
            tc.tile_pool(name="cflp", bufs=2) as cfl_pool,
            tc.tile_pool(name="sps", bufs=3, space="PSUM") as s_psum,
            tc.tile_pool(name="cps", bufs=1, space="PSUM") as c_psum,
            tc.tile_pool(name="yps", bufs=4, space="PSUM") as y_psum,
        ):
            at_sb = const_pool.tile([T, H * T], f32r)
            w_sb = const_pool.tile([T, H], f32r)
            gv_sb = const_pool.tile([1, H * T], f32r)
            abt_sb = const_pool.tile([KB, H * KB], f32r)
            nc.sync.dma_start(out=at_sb[:], in_=at_in[:])
            nc.sync.dma_start(out=w_sb[:], in_=w_in[:])
            nc.sync.dma_start(out=gv_sb[:], in_=gv_in[:])
            nc.sync.dma_start(out=abt_sb[:], in_=abt_in[:])

            xt = [None] * H      # per-head X tiles [128, 4096], free = (block, d)
            yt = [None] * H      # per-head output staging [128, 4096]
            s32 = [None] * H     # S as [KB, D]
            c32 = [None] * H     # C as [KB, D]
            cfl = [None] * H     # C_flat [1, KB*D]

            def stage_in(h):
                xt[h] = x_pool.tile([T, S], f32r, name=f"xt{h}", tag="xt")
                src = x_in[h].rearrange("(hf k p) d -> hf p k d", k=KB // 2, p=T)
                for hf in range(2):
                    dst = xt[h][:, hf * 2048 : (hf + 1) * 2048].rearrange(
                        "p (k d) -> p k d", d=D
                    )
                    nc.sync.dma_start(out=dst, in_=src[hf])

            def stage_s(h):
                s32[h] = small_pool.tile([KB, D], f32r, name=f"s32{h}", tag="s32")
                s_fl = sstage_pool.tile([1, KB * D], f32r, name="sfl", tag="sfl")
                for t in range(NT):
                    s_ps = s_psum.tile([1, TILE], mybir.dt.float32, name="sps", tag="sps")
                    nc.tensor.matmul(
                        s_ps[:],
                        w_sb[:, h : h + 1],
                        xt[h][:, t * TILE : (t + 1) * TILE],
                        start=True,
                        stop=True,
                    )
                    nc.scalar.copy(
                        out=s_fl[0:1, t * TILE : (t + 1) * TILE], in_=s_ps[:]
                    )
                nc.gpsimd.dma_start(out=s32[h][:], in_=s_fl[:])

            def stage_c(h):
                c_ps = c_psum.tile([KB, D], mybir.dt.float32, name="cps", tag="cps")
                nc.tensor.matmul(
                    c_ps[:],
                    abt_sb[:, h * KB : (h + 1) * KB],
                    s32[h][:],
                    start=True,
                    stop=True,
                )
                c32[h] = small_pool.tile([KB, D], f32r, name=f"c32{h}", tag="c32")
                nc.scalar.copy(out=c32[h][:], in_=c_ps[:])
                cfl[h] = cfl_pool.tile([1, KB * D], f32r, name=f"cf{h}", tag="cf")
                nc.gpsimd.dma_start(out=cfl[h][:], in_=c32[h][:])

            def stage_b(h):
                yt[h] = out_pool.tile([T, S], mybir.dt.float32, name=f"yt{h}", tag="yt")
                for t in range(NT):
                    y_ps = y_psum.tile(
                        [T, TILE], mybir.dt.float32, name="yps", tag="yps"
                    )
                    nc.tensor.matmul(
                        y_ps[:],
                        at_sb[:, h * T : (h + 1) * T],
                        xt[h][:, t * TILE : (t + 1) * TILE],
                        start=True,
                        stop=(mode == "nocarry"),
                    )
                    if mode != "nocarry":
                        nc.tensor.matmul(
                            y_ps[:],
                            gv_sb[0:1, h * T : (h + 1) * T],
                            cfl[h][0:1, t * TILE : (t + 1) * TILE],
                            start=False,
                            stop=True,
                        )
                    nc.vector.tensor_copy(
                        out=yt[h][:, t * TILE : (t + 1) * TILE], in_=y_ps[:]
                    )
                    if t % 2 == 1 and mode != "computeonly":
                        q = t // 2
                        dst = y_out[h].rearrange("(q k p) d -> q p k d", k=NT, p=T)[q]
                        src = yt[h][:, q * 1024 : (q + 1) * 1024].rearrange(
                            "p (k d) -> p k d", d=D
                        )
                        nc.scalar.dma_start(out=dst, in_=src)

            def stage_dma_out(h):
                # store xt straight back (bitcast f32r view of y): DMA floor probe
                for q in range(4):
                    dst = y_out[h].rearrange("(q k p) d -> q p k d", k=NT, p=T)[
                        q
                    ].bitcast(f32r)
                    src = xt[h][:, q * 1024 : (q + 1) * 1024].rearrange(
                        "p (k d) -> p k d", d=D
                    )
                    nc.scalar.dma_start(out=dst, in_=src)

            if mode == "computeonly":
                xconst = const_pool.tile([T, S], f32r)
                nc.vector.memset(xconst[:].bitcast(f32), 0.125)

                def stage_in(h):  # noqa: F811
                    xt[h] = xconst

                def _no_store(h, q):
                    return

            loop = tc.For_i(0, repeat, 1) if repeat > 1 else contextlib.nullcontext()
            with loop:
                if mode == "dmaonly":
                    for i in range(H):
                        stage_in(i)
                        stage_dma_out(i)
                elif mode == "nocarry":
                    for i in range(H + 1):
                        if i < H:
                            stage_in(i)
                        if 0 <= i - 1 < H:
                            stage_b(i - 1)
                else:
                    for i in range(H + SKEW_B):
                        if i < H:
                            stage_in(i)
                            stage_s(i)
                        if 0 <= i - SKEW_C < H:
                            stage_c(i - SKEW_C)
                        if 0 <= i - SKEW_B < H:
                            stage_b(i - SKEW_B)

    nc.compile()
    return nc


def _constants(gamma):
    g = gamma.astype(np.float64)  # [H]
    i = np.arange(T)
    # A_h[i, s] = g^(i-s) for i>=s ; AT[s, h*T+i] = A_h[i, s]
    diff = i[:, None] - i[None, :]  # [i, s]
    at = np.zeros((T, H * T), np.float64)
    w = np.zeros((T, H), np.float64)
    gv = np.zeros((1, H * T), np.float64)
    abt = np.zeros((KB, H * KB), np.float64)
    k = np.arange(KB)
    kdiff = k[None, :] - k[:, None] - 1  # [j, k] -> k-1-j
    for h in range(H):
        gh = g[h]
        a_h = np.where(diff >= 0, gh ** np.maximum(diff, 0), 0.0)  # [i, s]
        at[:, h * T : (h + 1) * T] = a_h.T
        w[:, h] = gh ** (T - 1 - i)
        gv[0, h * T : (h + 1) * T] = gh ** (i + 1)
        G = gh ** T
        abt[:, h * KB : (h + 1) * KB] = np.where(
            kdiff >= 0, G ** np.maximum(kdiff, 0), 0.0
        )
    return (
        at.astype(np.float32),
        w.astype(np.float32),
        gv.astype(np.float32),
        abt.astype(np.float32),
    )


def _fast_callable(nc):
    """Cached jitted shard_map callable (avoids per-call retrace)."""
    import jax
    from jax.experimental.shard_map import shard_map
    from jax.sharding import Mesh, NamedSharding, PartitionSpec
    from concourse import bass2jax, mybir

    bass2jax.install_neuronx_cc_hook()
    partition_name = nc.partition_id_tensor.name if nc.partition_id_tensor else None
    in_names, out_names, out_avals, zero_outs = [], [], [], []
    for alloc in nc.m.functions[0].allocations:
        if not isinstance(alloc, mybir.MemoryLocationSet):
            continue
        name = alloc.memorylocations[0].name
        if alloc.kind == "ExternalInput":
            if name != partition_name:
                in_names.append(name)
        elif alloc.kind == "ExternalOutput":
            shape = tuple(alloc.tensor_shape)
            dtype = mybir.dt.np(alloc.dtype)
            out_avals.append(jax.core.ShapedArray(shape, dtype))
            out_names.append(name)
            zero_outs.append(np.zeros(shape, dtype))
    n_params = len(in_names)
    all_in = list(in_names) + list(out_names)
    if partition_name is not None:
        all_in.append(partition_name)

    def _body(*args):
        operands = list(args)
        if partition_name is not None:
            operands.append(bass2jax.partition_id_tensor())
        return tuple(
            bass2jax._bass_exec_p.bind(
                *operands,
                out_avals=tuple(out_avals),
                in_names=tuple(all_in),
                out_names=tuple(out_names),
                lowering_input_output_aliases=(),
                sim_require_finite=True,
                sim_require_nnan=True,
                nc=nc,
            )
        )

    devices = jax.devices()[:B]
    mesh = Mesh(np.asarray(devices), ("core",))
    specs = (PartitionSpec("core"),)
    f = jax.jit(
        shard_map(
            _body,
            mesh=mesh,
            in_specs=specs * (n_params + len(out_names)),
            out_specs=specs * len(out_names),
            check_rep=False,
        ),
        keep_unused=True,
    )
    sharding = NamedSharding(mesh, PartitionSpec("core"))
    dev_zero = [
        jax.device_put(np.zeros((B * z.shape[0], *z.shape[1:]), z.dtype), sharding)
        for z in zero_outs
    ]
    return f, in_names, out_names, out_avals, sharding, dev_zero


def _run_fast(nc, in_maps):
    import jax

    if "fast" not in _CACHE:
        _CACHE["fast"] = _fast_callable(nc)
    f, in_names, out_names, out_avals, sharding, dev_zero = _CACHE["fast"]
    concat_in = [
        jax.device_put(
            np.concatenate([np.asarray(m[nm]) for m in in_maps], axis=0), sharding
        )
        for nm in in_names
    ]
    outs = f(*concat_in, *dev_zero)
    return [
        {
            nm: np.asarray(outs[i]).reshape(B, *out_avals[i].shape)[c]
            for i, nm in enumerate(out_names)
        }
        for c in range(B)
    ]


def _run(tensor, gamma, trace=False, repeat=1):
    from concourse.bass_utils import run_bass_kernel_spmd

    key = f"nc{repeat}"
    if key not in _CACHE:
        _CACHE[key] = _build(repeat)
    nc = _CACHE[key]

    at, w, gv, abt = _constants(np.asarray(gamma))
    tensor = np.asarray(tensor, dtype=np.float32)
    in_maps = [
        {
            "x": np.ascontiguousarray(tensor[c]),
            "at": at,
            "w": w,
            "gv": gv,
            "abt": abt,
        }
        for c in range(B)
    ]
    if repeat == 1 and not trace:
        try:
            results = _run_fast(nc, in_maps)
            y = np.stack([results[c]["y"] for c in range(B)], axis=0)
            return y, None
        except Exception:
            pass  # fall back to the reference path below
    res = run_bass_kernel_spmd(nc, in_maps, core_ids=list(range(B)), trace=trace)
    y = np.stack([res.results[c]["y"] for c in range(B)], axis=0)
    return y, res


def kernel(tensor, gamma):
    try:
        y, _ = _run(tensor, gamma)
    except Exception:
        # transient device/pool errors: clear cached state and retry once
        _CACHE.clear()
        y, _ = _run(tensor, gamma)
    return y



# revision 6
# speedup vs baseline: 40.2308x; 40.2308x over previous
"""Discounted cumsum (B,H,S,D)=(8,16,4096,128), gamma per head, scan along S.

Strategy: batch-parallel across 8 NeuronCores (1 batch each, all 16 heads).
HBM traffic is the roofline, so I/O is bf16 (rel-err budget 2e-2; bf16 I/O
lands ~1e-3) and the host pre/post-transposes so every device DMA moves
fully contiguous 8KB partition lines.

Per head (X laid out [T=128 part = pos-in-block, 4096 free = 32 blocks x 128 d]):
  - s-pass: 8 matmuls w^T X_t -> [1,512] PSUM (block discounted sums),
            copies (DVE/Act alternating) -> s_fl [1, 4096] -> gpsimd reshape
            to s32 [32, 128]
  - carry:  one [32x32] matmul  r = g*AB @ s
  - row0:   gpsimd DMA accum-add r into row 0 of X  (x'_0 = x_0 + g*c folds
            the carry into the next pass)
  - y-pass: 8 matmuls A @ X'_t -> PSUM -> bf16 staging -> one 1MiB DMA out
All matmul operands bf16, accumulation fp32.
"""
import sys

sys.path.insert(0, "/opt/trn_rl_repo")
import numpy as np

B, H, S, D = 8, 16, 4096, 128
T = 128          # block length along S
KB = S // T      # 32 blocks per head
TILE = 4 * T     # 512 free columns = 4 blocks per matmul
NT = S // TILE   # 8 tiles per head

_CACHE = {}


def _build(repeat=1):
    import contextlib

    import concourse.bacc as bacc
    import concourse.tile as tile
    from concourse import mybir

    f32 = mybir.dt.float32
    bf16 = mybir.dt.bfloat16

    nc = bacc.Bacc("TRN2", target_bir_lowering=False, debug=False)

    x_in = nc.declare_dram_parameter("x", [H, T, S], bf16, isOutput=False)
    at_in = nc.declare_dram_parameter("at", [T, H * T], bf16, isOutput=False)
    w_in = nc.declare_dram_parameter("w", [T, H], bf16, isOutput=False)
    aug_in = nc.declare_dram_parameter("aug", [KB, H * KB], bf16, isOutput=False)
    y_out = nc.declare_dram_parameter("y", [H, T, S], bf16, isOutput=True)

    with tile.TileContext(nc) as tc:
        with (
            tc.tile_pool(name="const", bufs=1) as const_pool,
            tc.tile_pool(name="xp", bufs=5) as x_pool,
            tc.tile_pool(name="op", bufs=3) as out_pool,
            tc.tile_pool(name="sflp", bufs=2) as sfl_pool,
            tc.tile_pool(name="sxp", bufs=2) as sx_pool,
            tc.tile_pool(name="rp", bufs=2) as r_pool,
            tc.tile_pool(name="sps", bufs=2, space="PSUM") as s_psum,
            tc.tile_pool(name="rps", bufs=1, space="PSUM") as r_psum,
            tc.tile_pool(name="yps", bufs=4, space="PSUM") as y_psum,
        ):
            at_sb = const_pool.tile([T, H * T], bf16)
            w_sb = const_pool.tile([T, H], bf16)
            aug_sb = const_pool.tile([KB, H * KB], bf16)
            nc.sync.dma_start(out=at_sb[:], in_=at_in[:])
            nc.sync.dma_start(out=w_sb[:], in_=w_in[:])
            nc.sync.dma_start(out=aug_sb[:], in_=aug_in[:])

            xt = [None] * H      # per-head X tiles [128, 4096] bf16
            yt = [None] * H      # per-head output staging [128, 4096] bf16
            s32 = [None] * H     # block sums as [KB, D]

            def stage_in(h):
                xt[h] = x_pool.tile([T, S], bf16, name=f"xt{h}", tag="xt")
                nc.sync.dma_start(out=xt[h][:], in_=x_in[h])

            def stage_s(h):
                s_fl = sfl_pool.tile([1, S], bf16, name="sfl", tag="sfl")
                for t in range(NT):
                    s_ps = s_psum.tile([1, TILE], f32, name="sps", tag="sps")
                    nc.tensor.matmul(
                        s_ps[:],
                        w_sb[:, h : h + 1],
                        xt[h][:, t * TILE : (t + 1) * TILE],
                        start=True,
                        stop=True,
                    )
                    if t % 2 == 0:
                        nc.vector.tensor_copy(
                            out=s_fl[0:1, t * TILE : (t + 1) * TILE], in_=s_ps[:]
                        )
                    else:
                        nc.scalar.copy(
                            out=s_fl[0:1, t * TILE : (t + 1) * TILE], in_=s_ps[:]
                        )
                s32[h] = sx_pool.tile([KB, D], bf16, name=f"s32{h}", tag="s32")
                # shape-mismatched APs (no rearrange): rearranged SWDGE
                # descriptors scatter wrong on HW (sim models them fine)
                nc.gpsimd.dma_start(out=s32[h][:], in_=s_fl[:])

            def stage_r(h):
                # r = g*AB @ s : carry scaled into row-0 units
                r_ps = r_psum.tile([KB, D], f32, name="rps", tag="rps")
                nc.tensor.matmul(
                    r_ps[:],
                    aug_sb[:, h * KB : (h + 1) * KB],
                    s32[h][:],
                    start=True,
                    stop=True,
                )
                r32 = r_pool.tile([KB, D], bf16, name="r32", tag="r32")
                nc.scalar.copy(out=r32[:], in_=r_ps[:])
                # accumulate r into row 0 of xt (x'_0 = x_0 + g*c)
                nc.gpsimd.dma_start(
                    out=xt[h][0:1, :],
                    in_=r32[:],
                    accum_op=mybir.AluOpType.add,
                )

            def stage_y(h):
                yt[h] = out_pool.tile([T, S], bf16, name=f"yt{h}", tag="yt")
                for t in range(NT):
                    y_ps = y_psum.tile([T, TILE], f32, name="yps", tag="yps")
                    nc.tensor.matmul(
                        y_ps[:],
                        at_sb[:, h * T : (h + 1) * T],
                        xt[h][:, t * TILE : (t + 1) * TILE],
                        start=True,
                        stop=True,
                    )
                    if t % 2 == 0:
                        nc.vector.tensor_copy(
                            out=yt[h][:, t * TILE : (t + 1) * TILE], in_=y_ps[:]
                        )
                    else:
                        nc.scalar.copy(
                            out=yt[h][:, t * TILE : (t + 1) * TILE], in_=y_ps[:]
                        )
                nc.sync.dma_start(out=y_out[h], in_=yt[h][:])

            loop = tc.For_i(0, repeat, 1) if repeat > 1 else contextlib.nullcontext()
            with loop:
                for i in range(H + 3):
                    if i < H:
                        stage_in(i)
                    if 0 <= i - 1 < H:
                        stage_s(i - 1)
                    if 0 <= i - 2 < H:
                        stage_r(i - 2)
                    if 0 <= i - 3 < H:
                        stage_y(i - 3)

    nc.compile()
    return nc


def _constants(gamma):
    from ml_dtypes import bfloat16

    g = gamma.astype(np.float64)  # [H]
    i = np.arange(T)
    diff = i[:, None] - i[None, :]  # [i, s]
    at = np.zeros((T, H * T), np.float64)
    w = np.zeros((T, H), np.float64)
    aug = np.zeros((KB, H * KB), np.float64)
    k = np.arange(KB)
    kdiff = k[None, :] - k[:, None] - 1  # [j, k] -> k-1-j
    for h in range(H):
        gh = g[h]
        a_h = np.where(diff >= 0, gh ** np.maximum(diff, 0), 0.0)  # [i, s]
        at[:, h * T : (h + 1) * T] = a_h.T
        w[:, h] = gh ** (T - 1 - i)
        G = gh ** T
        aug[:, h * KB : (h + 1) * KB] = gh * np.where(
            kdiff >= 0, G ** np.maximum(kdiff, 0), 0.0
        )
    return (
        at.astype(bfloat16),
        w.astype(bfloat16),
        aug.astype(bfloat16),
    )


def _prep_inputs(tensor, gamma):
    """Full f32 (B,H,S,D) -> per-core input maps in device layout (bf16)."""
    from ml_dtypes import bfloat16

    at, w, aug = _constants(np.asarray(gamma))
    xb = np.asarray(tensor).astype(bfloat16)
    # (H, S, D) -> (H, KB, T, D) -> (H, T, KB, D) -> (H, T, S)
    in_maps = [
        {
            "x": np.ascontiguousarray(
                xb[c].reshape(H, KB, T, D).transpose(0, 2, 1, 3)
            ).reshape(H, T, S),
            "at": at,
            "w": w,
            "aug": aug,
        }
        for c in range(B)
    ]
    return in_maps


def _postprocess(y_cores):
    """Per-core device outputs [H, T, S] bf16 -> full (B,H,S,D) f32."""
    y = np.stack(y_cores, axis=0)  # (B, H, T, S) bf16
    y = y.reshape(B, H, T, KB, D).transpose(0, 1, 3, 2, 4).reshape(B, H, S, D)
    return np.ascontiguousarray(y).astype(np.float32)


def _fast_callable(nc):
    """Cached jitted shard_map callable (avoids per-call retrace)."""
    import jax
    from jax.experimental.shard_map import shard_map
    from jax.sharding import Mesh, NamedSharding, PartitionSpec
    from concourse import bass2jax, mybir

    bass2jax.install_neuronx_cc_hook()
    partition_name = nc.partition_id_tensor.name if nc.partition_id_tensor else None
    in_names, out_names, out_avals, zero_outs = [], [], [], []
    for alloc in nc.m.functions[0].allocations:
        if not isinstance(alloc, mybir.MemoryLocationSet):
            continue
        name = alloc.memorylocations[0].name
        if alloc.kind == "ExternalInput":
            if name != partition_name:
                in_names.append(name)
        elif alloc.kind == "ExternalOutput":
            shape = tuple(alloc.tensor_shape)
            dtype = mybir.dt.np(alloc.dtype)
            out_avals.append(jax.core.ShapedArray(shape, dtype))
            out_names.append(name)
            zero_outs.append(np.zeros(shape, dtype))
    n_params = len(in_names)
    all_in = list(in_names) + list(out_names)
    if partition_name is not None:
        all_in.append(partition_name)

    def _body(*args):
        operands = list(args)
        if partition_name is not None:
            operands.append(bass2jax.partition_id_tensor())
        return tuple(
            bass2jax._bass_exec_p.bind(
                *operands,
                out_avals=tuple(out_avals),
                in_names=tuple(all_in),
                out_names=tuple(out_names),
                lowering_input_output_aliases=(),
                sim_require_finite=True,
                sim_require_nnan=True,
                nc=nc,
            )
        )

    devices = jax.devices()[:B]
    mesh = Mesh(np.asarray(devices), ("core",))
    specs = (PartitionSpec("core"),)
    f = jax.jit(
        shard_map(
            _body,
            mesh=mesh,
            in_specs=specs * (n_params + len(out_names)),
            out_specs=specs * len(out_names),
            check_rep=False,
        ),
        keep_unused=True,
    )
    sharding = NamedSharding(mesh, PartitionSpec("core"))
    dev_zero = [
        jax.device_put(np.zeros((B * z.shape[0], *z.shape[1:]), z.dtype), sharding)
        for z in zero_outs
    ]
    return f, in_names, out_names, out_avals, sharding, dev_zero


def _run_fast(nc, in_maps):
    import jax

    if "fast" not in _CACHE:
        _CACHE["fast"] = _fast_callable(nc)
    f, in_names, out_names, out_avals, sharding, dev_zero = _CACHE["fast"]
    concat_in = [
        jax.device_put(
            np.concatenate([np.asarray(m[nm]) for m in in_maps], axis=0), sharding
        )
        for nm in in_names
    ]
    outs = f(*concat_in, *dev_zero)
    return [
        {
            nm: np.asarray(outs[i]).reshape(B, *out_avals[i].shape)[c]
            for i, nm in enumerate(out_names)
        }
        for c in range(B)
    ]


def _run(tensor, gamma, trace=False, repeat=1):
    from concourse.bass_utils import run_bass_kernel_spmd

    key = f"nc{repeat}"
    if key not in _CACHE:
        _CACHE[key] = _build(repeat)
    nc = _CACHE[key]

    in_maps = _prep_inputs(tensor, gamma)
    if repeat == 1 and not trace:
        try:
            results = _run_fast(nc, in_maps)
            y = _postprocess([results[c]["y"] for c in range(B)])
            return y, None
        except Exception:
            pass  # fall back to the reference path below
    res = run_bass_kernel_spmd(nc, in_maps, core_ids=list(range(B)), trace=trace)
    y = _postprocess([res.results[c]["y"] for c in range(B)])
    return y, res


def kernel(tensor, gamma):
    try:
        y, _ = _run(tensor, gamma)
    except Exception:
        # transient device/pool errors: clear cached state and retry once
        _CACHE.clear()
        y, _ = _run(tensor, gamma)
    return y
